# revision 18
# baseline (speedup 1.0000x reference)
"""MinLSTM (B=16, T=2048, H=768, L=2) on 8 TRN2 NeuronCores.

Strategy: pure data-parallel over batch (2 sequences per core, no
collectives). Per core, per layer, the three gate GEMMs are computed in
transposed orientation gate^T[h, t] = W^T[d, h]^T-matmul x^T[d, t] so that
the time axis lands in the SBUF free dimension; the MinLSTM linear
recurrence h_t = fp_t*h_{t-1} + ip_t*ht_t then maps directly onto the
Vector engine's tensor_tensor_scan instruction (state = d0*state + d1),
chained across T-chunks via the previous chunk's last column.

Host side pre-transposes x -> x^T and W -> W^T (free: not on the device
clock), shards batch 2-per-core, and un-transposes the output.

Layout per core (all f32):
  xT   [2, 768, 2048]  input, layer-0 x^T
  wT   [2, 3, 768, 768] W^T per (layer, gate=f/i/h): [d, h]
  bias [2, 3, 768]
  h0   [2, 2, 768]
  outT [2, 768, 2048]  layer-1 h^T (final output)
  hlast[2, 2, 768]     h_T per layer
Layer-0 h^T goes through a DRAM scratch tile (SBUF can't hold it plus
both layers' weights).

Matmuls run as float32r (full PE rate at moving dim 512 vs 4x slower
plain fp32).
"""

import os

import ml_dtypes
import numpy as np

import concourse.bacc as bacc
import concourse.mybir as mybir
import concourse.tile as tile
from concourse.bass_utils import run_bass_kernel_spmd

B, T, H, L = 16, 2048, 768, 2
D = H
EPS = 1e-8
NCORES = 8
BPC = B // NCORES  # sequences per core
PT = 128
MD = D // PT  # 6 contraction tiles
MH = H // PT  # 6 output-row tiles
TC = 512      # time chunk (one PSUM bank at f32)
NT = T // TC

F32 = mybir.dt.float32
BF16 = mybir.dt.bfloat16
ADD = mybir.AluOpType.add
MULT = mybir.AluOpType.mult
SIGMOID = mybir.ActivationFunctionType.Sigmoid

_CACHE = {}
LAST_RESULTS = None


def _build(reps=1):
    nc = bacc.Bacc("TRN2", target_bir_lowering=False, debug=False)

    xT = nc.dram_tensor("xT", [BPC, D, T], BF16, kind="ExternalInput")
    h0 = nc.dram_tensor("h0", [BPC, L, H], F32, kind="ExternalInput")
    wT = nc.dram_tensor("wT", [L, 3, D, H], BF16, kind="ExternalInput")
    bias = nc.dram_tensor("bias", [L, 3, H], F32, kind="ExternalInput")
    outT = nc.dram_tensor("outT", [BPC, H, T], F32, kind="ExternalOutput")
    hlast = nc.dram_tensor("hlast", [BPC, L, H], F32, kind="ExternalOutput")

    with tile.TileContext(nc) as tc:
        with (
            tc.tile_pool(name="dram", bufs=1, space="DRAM") as dpool,
            tc.tile_pool(name="wpool", bufs=1) as wpool,
            tc.tile_pool(name="cpool", bufs=1) as cpool,
            tc.tile_pool(name="xpool", bufs=2) as xpool,
            tc.tile_pool(name="gpool", bufs=2) as gpool,
            tc.tile_pool(name="opool", bufs=2) as opool,
            tc.tile_pool(name="psum", bufs=2, space="PSUM") as psum,
        ):
            hmid = dpool.tile([BPC, H, T], BF16, name="hmid")

            # Weights for both layers stay resident (108KB/partition).
            w_tiles = {}
            for l in range(L):
                for g in range(3):
                    for d in range(MD):
                        wt = wpool.tile([PT, H], BF16, tag=f"w{l}{g}{d}", name=f"w{l}{g}{d}")
                        nc.sync.dma_start(wt[:], wT[l, g, d * PT:(d + 1) * PT, :])
                        w_tiles[(l, g, d)] = wt

            # Bias columns: [128, MH] per (layer, gate); column m serves h-tile m.
            b_tiles = {}
            for l in range(L):
                for g in range(3):
                    bt = cpool.tile([PT, MH], F32, tag=f"b{l}{g}", name=f"b{l}{g}")
                    nc.sync.dma_start(bt[:], bias[l, g].rearrange("(m p) -> p m", p=PT))
                    b_tiles[(l, g)] = bt

            # Initial hidden state columns per (seq, layer).
            h0_tiles = {}
            for b in range(BPC):
                for l in range(L):
                    t0 = cpool.tile([PT, MH], F32, tag=f"h0{b}{l}", name=f"h0{b}{l}")
                    nc.sync.dma_start(t0[:], h0[b, l].rearrange("(m p) -> p m", p=PT))
                    h0_tiles[(b, l)] = t0

            for _rep in range(reps):
              for l in range(L):
                src = xT if l == 0 else hmid
                dst = hmid if l < L - 1 else outT
                for b in range(BPC):
                    prev = {}
                    for n in range(NT):
                        xin = []
                        for d in range(MD):
                            xt = xpool.tile([PT, TC], BF16, tag=f"x{d}", name=f"x{d}", bufs=3)
                            nc.sync.dma_start(
                                xt[:], src[b, d * PT:(d + 1) * PT, n * TC:(n + 1) * TC]
                            )
                            xin.append(xt)
                        for m in range(MH):
                            pf = psum.tile([PT, TC], F32, tag="pf", name="pf", bufs=3)
                            pi = psum.tile([PT, TC], F32, tag="pi", name="pi", bufs=3)
                            ph = psum.tile([PT, TC], F32, tag="ph", name="ph", bufs=2)
                            for g, pt in ((0, pf), (1, pi), (2, ph)):
                                for d in range(MD):
                                    nc.tensor.matmul(
                                        pt[:],
                                        w_tiles[(l, g, d)][:, m * PT:(m + 1) * PT],
                                        xin[d][:],
                                        start=(d == 0),
                                        stop=(d == MD - 1),
                                    )
                            f_t = gpool.tile([PT, TC], F32, tag="f", name="f_t")
                            i_t = gpool.tile([PT, TC], F32, tag="i", name="i_t")
                            nc.scalar.activation(
                                f_t[:], pf[:], SIGMOID, bias=b_tiles[(l, 0)][:, m:m + 1]
                            )
                            nc.scalar.activation(
                                i_t[:], pi[:], SIGMOID, bias=b_tiles[(l, 1)][:, m:m + 1]
                            )
                            # den = (f + eps) + i
                            den = gpool.tile([PT, TC], F32, tag="den", name="den")
                            nc.vector.scalar_tensor_tensor(
                                den[:], f_t[:], EPS, i_t[:], op0=ADD, op1=ADD
                            )
                            rd = gpool.tile([PT, TC], F32, tag="rd", name="rd")
                            nc.vector.reciprocal(rd[:], den[:])
                            # u = (psum_h + bh) * i
                            u = gpool.tile([PT, TC], F32, tag="u", name="u")
                            nc.vector.scalar_tensor_tensor(
                                u[:], ph[:], b_tiles[(l, 2)][:, m:m + 1], i_t[:],
                                op0=ADD, op1=MULT,
                            )
                            # fp = f * rd, bb = u * rd  (gpsimd: keep DVE free for scans)
                            fp = gpool.tile([PT, TC], F32, tag="fp", name="fp")
                            nc.gpsimd.tensor_mul(fp[:], f_t[:], rd[:])
                            bb = gpool.tile([PT, TC], F32, tag="bb", name="bb")
                            nc.gpsimd.tensor_mul(bb[:], u[:], rd[:])
                            ho = opool.tile([PT, TC], F32, tag=f"ho{m}", name=f"ho{m}")
                            init = (
                                h0_tiles[(b, l)][:, m:m + 1]
                                if n == 0
                                else prev[m][:, TC - 1:TC]
                            )
                            nc.vector.tensor_tensor_scan(
                                ho[:], fp[:], bb[:], init, op0=MULT, op1=ADD
                            )
                            prev[m] = ho
                            if l < L - 1:
                                # hmid is bf16; downcast on ACT then store cast-free.
                                ho_bf = gpool.tile([PT, TC], BF16, tag="hobf", name="ho_bf")
                                nc.scalar.copy(ho_bf[:], ho[:])
                                nc.sync.dma_start(
                                    dst[b, m * PT:(m + 1) * PT, n * TC:(n + 1) * TC],
                                    ho_bf[:],
                                )
                            else:
                                nc.sync.dma_start(
                                    dst[b, m * PT:(m + 1) * PT, n * TC:(n + 1) * TC],
                                    ho[:],
                                )
                            if n == NT - 1:
                                nc.sync.dma_start(
                                    hlast[b, l, m * PT:(m + 1) * PT], ho[:, TC - 1:TC]
                                )

    nc.compile()
    return nc


def _get_nc():
    if "nc" not in _CACHE:
        _CACHE["nc"] = _build()
    return _CACHE["nc"]


def kernel(x, h0, Wf, bf, Wi, bi, Wh, bh):
    global LAST_RESULTS
    x = np.asarray(x, dtype=np.float32)
    h0 = np.ascontiguousarray(np.asarray(h0, dtype=np.float32))
    xT = np.ascontiguousarray(x.transpose(0, 2, 1)).astype(ml_dtypes.bfloat16)
    # wT[l, g] = W_g[l]^T  ([d, h])
    wT = np.ascontiguousarray(
        np.stack([np.asarray(Wf), np.asarray(Wi), np.asarray(Wh)], axis=1)
        .astype(np.float32)
        .transpose(0, 1, 3, 2)
    ).astype(ml_dtypes.bfloat16)  # [L, 3, D, H]
    bias = np.ascontiguousarray(
        np.stack([np.asarray(bf), np.asarray(bi), np.asarray(bh)], axis=1).astype(
            np.float32
        )
    )  # [L, 3, H]

    nc = _get_nc()
    in_maps = [
        {
            "xT": xT[c * BPC:(c + 1) * BPC],
            "h0": h0[c * BPC:(c + 1) * BPC],
            "wT": wT,
            "bias": bias,
        }
        for c in range(NCORES)
    ]
    trace = bool(int(os.environ.get("KERNEL_TRACE", "0")))
    res = run_bass_kernel_spmd(nc, in_maps, core_ids=list(range(NCORES)), trace=trace)
    LAST_RESULTS = res
    outT = np.concatenate([r["outT"] for r in res.results], axis=0)  # [B, H, T]
    outs = np.ascontiguousarray(outT.transpose(0, 2, 1))  # [B, T, H]
    h_outs = np.concatenate([r["hlast"] for r in res.results], axis=0)  # [B, L, H]
    return outs, h_outs


# revision 54
# speedup vs baseline: 1.2999x; 1.2999x over previous
"""MinLSTM (B=16, T=2048, H=768, L=2) on 8 TRN2 NeuronCores.

Sharding: pure data-parallel over batch (2 sequences per core, no
collectives). Per core, per layer, the three gate GEMMs run in transposed
orientation gate^T[h, t] = (W^T)^T @ x^T so the time axis lands in the
SBUF free dimension; the MinLSTM recurrence h_t = fp_t*h_{t-1} +
ip_t*ht_t then maps directly onto the Vector engine's tensor_tensor_scan
instruction (state = d0*state + d1), chained across T-chunks via the
previous chunk's last column.

Performance structure (cost-model-guided):
- GEMM operands are bf16 (full PE rate, 1 cycle/row; fp32 would be 4x
  slower, fp8 fails accuracy). Host pre-casts x^T and W^T to bf16.
- Layer-0's output stays resident in SBUF ([128, MH, T] bf16 per
  sequence) and feeds layer-1's GEMM directly - no DRAM round-trip.
- All three PSUM banks drain through the Scalar engine (sigmoid+bias x2,
  identity+bias) so matmul WAW slot-reuse never waits on DVE.
- The whole normalize/scan chain (den, reciprocal, fp, bb, scan) stays
  on the in-order DVE queue - no cross-engine bounce in the per-unit
  critical path; u = ht*i runs on gpsimd in parallel.
- DMAs are batched (one per weight gate-third, x chunk, store chunk) and
  split across queues: loads on SP, weights on ACT, with one-chunk
  lookahead so in-order queues never block a load behind a scan-gated
  store.
- First/last phases use tapered chunk widths to shorten pipeline fill
  and drain.

Host side pre-transposes x -> x^T [B, D, T] and W -> W^T [L, 3, D, H],
pre-tiles h0/bias to [128, ..., MH] column layouts, shards batch
2-per-core, and un-transposes outputs.
"""

import os

import ml_dtypes
import numpy as np

import concourse.bacc as bacc
import concourse.mybir as mybir
import concourse.tile as tile
from concourse.bass_utils import run_bass_kernel_spmd

B, T, H, L = 16, 2048, 768, 2
D = H
EPS = 1e-8
NCORES = 8
BPC = B // NCORES  # sequences per core
PT = 128
MD = D // PT  # 6 contraction tiles
MH = H // PT  # 6 output-row tiles
TC = 512      # time chunk (one PSUM bank at f32)
NT = T // TC

F32 = mybir.dt.float32
BF16 = mybir.dt.bfloat16
ADD = mybir.AluOpType.add
MULT = mybir.AluOpType.mult
DIV = mybir.AluOpType.divide
SIGMOID = mybir.ActivationFunctionType.Sigmoid
IDENT = mybir.ActivationFunctionType.Identity

_CACHE = {}
TAPER = [512, 512, 512, 384, 128]
STAPER = [256, 256, 512, 512, 512]
LAST_RESULTS = None


def _build(reps=1, kdep=MD, gbufs=2, obufs=2, xbufs=2, wq="scalar", pbufs=(3, 3, 2), fbufs=4, fpe="vector", bbe="vector"):
    nc = bacc.Bacc("TRN2", target_bir_lowering=False, debug=False)
    FP_ENG = getattr(nc, fpe)
    BB_ENG = getattr(nc, bbe)

    xT = nc.dram_tensor("xT", [BPC, D, T], BF16, kind="ExternalInput")
    # host supplies h0/bias pre-tiled as [..., PT, MH] so each lands as a
    # [128, cols] SBUF tile with one contiguous run per partition row
    h0 = nc.dram_tensor("h0", [PT, BPC, L, MH], F32, kind="ExternalInput")
    wT = nc.dram_tensor("wT", [L, 3, D, H], BF16, kind="ExternalInput")
    bias = nc.dram_tensor("bias", [PT, L, 3, MH], F32, kind="ExternalInput")
    outT = nc.dram_tensor("outT", [BPC, H, T], F32, kind="ExternalOutput")
    hlast = nc.dram_tensor("hlast", [PT, BPC, L, MH], F32, kind="ExternalOutput")

    with tile.TileContext(nc) as tc:
        with (
            tc.tile_pool(name="wpool", bufs=1) as wpool,
            tc.tile_pool(name="cpool", bufs=1) as cpool,
            tc.tile_pool(name="xpool", bufs=xbufs) as xpool,
            tc.tile_pool(name="gpool", bufs=gbufs) as gpool,
            tc.tile_pool(name="opool", bufs=obufs) as opool,
            tc.tile_pool(name="psum", bufs=2, space="PSUM") as psum,
        ):
            # One [128, MD, H] weight tile per (layer, gate); a single DMA
            # each. Layer-1 loads are deferred past the first phase so they
            # don't steal DMA bandwidth at startup.
            w_tiles = {}
            for l in range(L):
                for g in range(3):
                    w_tiles[(l, g)] = wpool.tile(
                        [PT, MD, H], BF16, tag=f"w{l}{g}", name=f"w{l}{g}"
                    )

            def load_weights(l):
                for g in range(3):
                    wr = wT[l, g].rearrange("(d p) h -> p d h", p=PT)
                    for d0 in range(0, MD, 2):
                        getattr(nc, wq).dma_start(
                            w_tiles[(l, g)][:, d0:d0 + 2, :], wr[:, d0:d0 + 2, :]
                        )

            load_weights(0)

            # layer-0 output stays resident in SBUF ([128, MH, T] bf16 per
            # sequence): layer-1 reads it directly as the GEMM rhs — no DRAM
            # round-trip at all.
            hfull = {
                b: wpool.tile([PT, MH, T], BF16, tag=f"hf{b}", name=f"hf{b}")
                for b in range(BPC)
            }

            # All biases in one [128, L*3*MH] tile; column serves (l, g, m).
            ball = cpool.tile([PT, L * 3 * MH], F32, name="ball")
            nc.sync.dma_start(ball[:], bias.rearrange("p l g m -> p (l g m)"))

            def bcol(l, g, m):
                c = (l * 3 + g) * MH + m
                return ball[:, c:c + 1]

            # All h0 columns in one [128, BPC*L*MH] tile.
            h0all = cpool.tile([PT, BPC * L * MH], F32, name="h0all")
            nc.sync.dma_start(h0all[:], h0.rearrange("p b l m -> p (b l m)"))

            def h0col(b, l, m):
                c = (b * L + l) * MH + m
                return h0all[:, c:c + 1]

            # Flat work-item stream with one-chunk DMA lookahead: each x
            # load is emitted BEFORE the previous chunk's store so the
            # in-order SP queue never blocks a load behind a scan-gated
            # store. Stores and loads both live on the SP queue; the ACT
            # queue carries only PSUM-drain ops.
            def chunks_for(l, b, rep):
                last = rep == reps - 1 and l == L - 1 and b == BPC - 1
                first = rep == 0 and l == 0 and b == 0
                if last:
                    widths = TAPER
                elif first:
                    widths = STAPER
                else:
                    widths = [TC] * NT
                out, t0 = [], 0
                for w in widths:
                    out.append((t0, w))
                    t0 += w
                return out

            items = [
                (l, b, n, t0, tcw)
                for _rep in range(reps)
                for l in range(L)
                for b in range(BPC)
                for n, (t0, tcw) in enumerate(chunks_for(l, b, _rep))
            ]

            def out_ap(b):
                return outT[b].rearrange("(m p) t -> p m t", p=PT)

            def load_chunk(l, b, n, t0, tcw):
                if l != 0:
                    return None  # layer-1 rhs comes straight from hfull
                xc = xpool.tile([PT, MD, tcw], BF16, tag="x", name="xc", padded_shape=[PT, MD, TC])
                nc.sync.dma_start(
                    xc[:], xT[b].rearrange("(d p) t -> p d t", p=PT)[:, :, t0:t0 + tcw]
                )
                return xc

            pending = {0: load_chunk(*items[0])}
            prev_w = TC
            for idx, (l, b, n, t0, tcw) in enumerate(items):
                if idx + 1 < len(items):
                    pending[idx + 1] = load_chunk(*items[idx + 1])
                if idx == NT:
                    load_weights(1)
                xc = pending.pop(idx)
                # layer-0 writes its scan output straight into hfull (bf16);
                # layer-1 gets a rotating f32 tile headed for outT
                if l < L - 1:
                    hoall = None
                else:
                    hoall = opool.tile(
                        [PT, MH, tcw], F32, tag="ho", name="hoall",
                        padded_shape=[PT, MH, TC]
                    )
                for m in range(MH):
                    pf = psum.tile([PT, tcw], F32, tag="pf", name="pf", bufs=pbufs[0], padded_shape=[PT, TC])
                    pi = psum.tile([PT, tcw], F32, tag="pi", name="pi", bufs=pbufs[1], padded_shape=[PT, TC])
                    ph = psum.tile([PT, tcw], F32, tag="ph", name="ph", bufs=pbufs[2], padded_shape=[PT, TC])
                    for g, pt in ((0, pf), (1, pi), (2, ph)):
                        for d in range(kdep):
                            rhs = (
                                xc[:, d, :] if l == 0
                                else hfull[b][:, d, t0:t0 + tcw]
                            )
                            nc.tensor.matmul(
                                pt[:],
                                w_tiles[(l, g)][:, d, m * PT:(m + 1) * PT],
                                rhs,
                                start=(d == 0),
                                stop=(d == kdep - 1),
                            )
                    # Drain all three PSUM banks via ACT (least-loaded
                    # engine) so matmul WAW waits never queue behind DVE.
                    f_t = gpool.tile([PT, tcw], F32, tag="f", name="f_t", bufs=fbufs, padded_shape=[PT, TC])
                    i_t = gpool.tile([PT, tcw], F32, tag="i", name="i_t", bufs=fbufs, padded_shape=[PT, TC])
                    ht_t = gpool.tile([PT, tcw], F32, tag="ht", name="ht_t", bufs=fbufs, padded_shape=[PT, TC])
                    nc.scalar.activation(f_t[:], pf[:], SIGMOID, bias=bcol(l, 0, m))
                    nc.scalar.activation(i_t[:], pi[:], SIGMOID, bias=bcol(l, 1, m))
                    nc.scalar.activation(ht_t[:], ph[:], IDENT, bias=bcol(l, 2, m))
                    # den = (f + eps) + i; gate normalization via DVE divide
                    # (no reciprocal step -> shorter per-unit chain); u = ht*i
                    # runs on gpsimd in parallel with den/fp
                    den = gpool.tile([PT, tcw], F32, tag="den", name="den", padded_shape=[PT, TC])
                    nc.vector.scalar_tensor_tensor(
                        den[:], f_t[:], EPS, i_t[:], op0=ADD, op1=ADD
                    )
                    rd = gpool.tile([PT, tcw], F32, tag="rd", name="rd", padded_shape=[PT, TC])
                    nc.vector.reciprocal(rd[:], den[:])
                    u = gpool.tile([PT, tcw], F32, tag="u", name="u", padded_shape=[PT, TC])
                    nc.gpsimd.tensor_mul(u[:], ht_t[:], i_t[:])
                    fp = gpool.tile([PT, tcw], F32, tag="fp", name="fp", padded_shape=[PT, TC])
                    FP_ENG.tensor_mul(fp[:], f_t[:], rd[:])
                    bb = gpool.tile([PT, tcw], F32, tag="bb", name="bb", padded_shape=[PT, TC])
                    BB_ENG.tensor_mul(bb[:], u[:], rd[:])
                    if l < L - 1:
                        sout = hfull[b][:, m, t0:t0 + tcw]
                        init = h0col(b, l, m) if n == 0 else hfull[b][:, m, t0 - 1:t0]
                    else:
                        sout = hoall[:, m, :]
                        init = h0col(b, l, m) if n == 0 else prev[:, m, prev_w - 1:prev_w]
                    nc.vector.tensor_tensor_scan(
                        sout, fp[:], bb[:], init, op0=MULT, op1=ADD
                    )
                if l == L - 1:
                    prev = hoall
                    prev_w = tcw
                    nc.sync.dma_start(out_ap(b)[:, :, t0:t0 + tcw], hoall[:])
                if t0 + tcw == T:
                    # pull the last column through a tiny gpsimd copy (also
                    # upcasts bf16->f32 for layer 0), store on SP
                    last = hfull[b][:, :, T - 1] if l < L - 1 else hoall[:, :, tcw - 1]
                    hl_t = gpool.tile([PT, MH], F32, tag="hl", name="hl_t", bufs=2)
                    nc.gpsimd.tensor_copy(hl_t[:], last)
                    nc.sync.dma_start(hlast[:, b, l, :], hl_t[:])

    nc.compile()
    return nc


def _get_nc():
    if "nc" not in _CACHE:
        _CACHE["nc"] = _build()
    return _CACHE["nc"]


def kernel(x, h0, Wf, bf, Wi, bi, Wh, bh):
    global LAST_RESULTS
    x = np.asarray(x, dtype=np.float32)
    h0 = np.ascontiguousarray(np.asarray(h0, dtype=np.float32))
    xT = np.ascontiguousarray(x.transpose(0, 2, 1)).astype(ml_dtypes.bfloat16)
    # wT[l, g] = W_g[l]^T  ([d, h])
    wT = np.ascontiguousarray(
        np.stack([np.asarray(Wf), np.asarray(Wi), np.asarray(Wh)], axis=1)
        .astype(np.float32)
        .transpose(0, 1, 3, 2)
    ).astype(ml_dtypes.bfloat16)  # [L, 3, D, H]
    bias = np.ascontiguousarray(
        np.stack([np.asarray(bf), np.asarray(bi), np.asarray(bh)], axis=1)
        .astype(np.float32)
        .reshape(L, 3, MH, PT)
        .transpose(3, 0, 1, 2)
    )  # [PT, L, 3, MH]
    h0 = np.ascontiguousarray(h0.reshape(B, L, MH, PT).transpose(3, 0, 1, 2))

    nc = _get_nc()
    in_maps = [
        {
            "xT": xT[c * BPC:(c + 1) * BPC],
            "h0": h0[:, c * BPC:(c + 1) * BPC],
            "wT": wT,
            "bias": bias,
        }
        for c in range(NCORES)
    ]
    trace = bool(int(os.environ.get("KERNEL_TRACE", "0")))
    res = run_bass_kernel_spmd(nc, in_maps, core_ids=list(range(NCORES)), trace=trace)
    LAST_RESULTS = res
    outT = np.concatenate([r["outT"] for r in res.results], axis=0)  # [B, H, T]
    outs = np.ascontiguousarray(outT.transpose(0, 2, 1))  # [B, T, H]
    hl = np.concatenate([r["hlast"] for r in res.results], axis=1)  # [PT, B, L, MH]
    h_outs = np.ascontiguousarray(hl.transpose(1, 2, 3, 0).reshape(B, L, H))
    return outs, h_outs

